# revision 3
# baseline (speedup 1.0000x reference)
"""Trainium2 Bass kernel for nn_Decoder (dense_mlp).

Computation (reference):
    x   = z @ softplus(W_mix).T                     # [N, D]
    h1  = tanh(x[:, :, None] * W1 + b1)             # [N, D, H]
    h2  = tanh(einsum("ndh,dhk->ndk", h1, W2) + b2) # [N, D, H]
    out = einsum("ndh,dh->nd", h2, W3) + b3         # [N, D]

N=16384, L=16, D=128, H=64. Sharded data-parallel over N across 8 cores
(2048 samples/core); all weights replicated. b3 is added host-side (exact
same fp32 math as the reference's final add).

Device layout: "transposed" — activations live as [128 partitions, samples]
planes where the 128 partitions hold (2 channels x 64 hidden) for one
channel-pair p (channels 2p, 2p+1), 64 pairs total.

  Stage A: g1 = lhsA_p.T @ z_T      K=16 matmul; lhsA fuses softplus(W_mix)
           with W1 (host-precomputed), so g1[(c,h),n] = x[n,d]*W1[d,h].
  tanh1  : h1 = tanh(g1 + b1)       ScalarE, per-partition bias AP.
  Stage C: g2 = lhs2_p.T @ h1       K=128 matmul, lhs2_p = blockdiag(W2[2p],
           W2[2p+1]).
  tanh2  : h2 = tanh(g2 + b2)       ScalarE.
  Stage E: e = lhsE_p.T @ h2        [128,2]->[2,chunk] matmul with W3 folded
           in; matmul outputs can only land at PSUM partition 0 here, so
           pairs are processed in duos sharing a 2-bank PSUM tile, a DVE
           copy stages the 2x(2xchunk) rows to SBUF, and a scatter-DMA
           writes them to the right DRAM rows.

Pairs are emitted in duos (a, b) so the ScalarE queue alternates
tanh1(a), tanh1(b), tanh2(a), tanh2(b) — each op's producer matmul runs
during the previous ACT op, keeping ACT (the bottleneck engine) stall-free.
"""

import numpy as np

import concourse.bass as bass
import concourse.mybir as mybir
import concourse.tile as tile
from concourse import bacc
from concourse.bass_utils import run_bass_kernel_spmd

N_CORES = 8
N, L, D, H = 16384, 16, 128, 64
NC_SAMP = N // N_CORES          # 2048 samples per core
CHUNK = 512                     # free-dim tile (one PSUM bank of fp32)
NCHUNKS = NC_SAMP // CHUNK      # 4
NPAIR = D // 2                  # 64 channel pairs
NDUO = NPAIR // 2               # 32 duos

F32 = mybir.dt.float32


def _build_bass():
    nc = bacc.Bacc(None, target_bir_lowering=False)

    z_t = nc.dram_tensor("z_t", [L, NC_SAMP], F32, kind="ExternalInput")
    lhsA = nc.dram_tensor("lhsA", [L, NPAIR * 128], F32, kind="ExternalInput")
    lhs2 = nc.dram_tensor("lhs2", [128, NPAIR * 128], F32, kind="ExternalInput")
    lhsE = nc.dram_tensor("lhsE", [128, NPAIR * 2], F32, kind="ExternalInput")
    b1c = nc.dram_tensor("b1c", [128, NPAIR], F32, kind="ExternalInput")
    b2c = nc.dram_tensor("b2c", [128, NPAIR], F32, kind="ExternalInput")
    out_t = nc.dram_tensor("out_t", [128, NC_SAMP], F32, kind="ExternalOutput")

    with tile.TileContext(nc) as tc:
        with (
            tc.tile_pool(name="consts", bufs=1) as consts,
            tc.tile_pool(name="work", bufs=3) as work,
            tc.tile_pool(name="stage", bufs=3) as stage,
            tc.tile_pool(name="psA", bufs=2, space="PSUM") as psA,
            tc.tile_pool(name="psC", bufs=2, space="PSUM") as psC,
            tc.tile_pool(name="psE", bufs=2, space="PSUM") as psE,
        ):
            z_sb = consts.tile([L, NC_SAMP], F32)
            lhsA_sb = consts.tile([L, NPAIR * 128], F32)
            lhs2_sb = consts.tile([128, NPAIR * 128], F32)
            lhsE_sb = consts.tile([128, NPAIR * 2], F32)
            b1_sb = consts.tile([128, NPAIR], F32)
            b2_sb = consts.tile([128, NPAIR], F32)

            nc.sync.dma_start(out=z_sb[:], in_=z_t[:])
            nc.sync.dma_start(out=lhsA_sb[:], in_=lhsA[:])
            nc.sync.dma_start(out=lhs2_sb[:], in_=lhs2[:])
            nc.sync.dma_start(out=lhsE_sb[:], in_=lhsE[:])
            nc.sync.dma_start(out=b1_sb[:], in_=b1c[:])
            nc.sync.dma_start(out=b2_sb[:], in_=b2c[:])

            def mm_a(g1, p, ns):
                nc.tensor.matmul(
                    g1[:], lhsA_sb[:, p * 128:(p + 1) * 128], z_sb[:, ns],
                    start=True, stop=True)

            def mm_c(g2, h1, p):
                nc.tensor.matmul(
                    g2[:], lhs2_sb[:, p * 128:(p + 1) * 128], h1[:],
                    start=True, stop=True)

            def tanh(h, g, bias_sb, p):
                nc.scalar.activation(
                    h[:], g[:], mybir.ActivationFunctionType.Tanh,
                    bias=bias_sb[:, p:p + 1])

            for i in range(NCHUNKS):
                ns = slice(i * CHUNK, (i + 1) * CHUNK)
                for t in range(NDUO):
                    pa, pb = 2 * t, 2 * t + 1
                    eacc = psE.tile([128, 2, CHUNK], F32)
                    g1a = psA.tile([128, CHUNK], F32, tag="g1")
                    g1b = psA.tile([128, CHUNK], F32, tag="g1")
                    mm_a(g1a, pa, ns)
                    mm_a(g1b, pb, ns)
                    h1a = work.tile([128, CHUNK], F32, tag="h1")
                    h1b = work.tile([128, CHUNK], F32, tag="h1")
                    tanh(h1a, g1a, b1_sb, pa)
                    tanh(h1b, g1b, b1_sb, pb)
                    g2a = psC.tile([128, CHUNK], F32, tag="g2")
                    g2b = psC.tile([128, CHUNK], F32, tag="g2")
                    mm_c(g2a, h1a, pa)
                    mm_c(g2b, h1b, pb)
                    h2a = work.tile([128, CHUNK], F32, tag="h2")
                    h2b = work.tile([128, CHUNK], F32, tag="h2")
                    tanh(h2a, g2a, b2_sb, pa)
                    tanh(h2b, g2b, b2_sb, pb)
                    nc.tensor.matmul(
                        eacc[0:2, 0, :], lhsE_sb[:, 2 * pa:2 * pa + 2], h2a[:],
                        start=True, stop=True)
                    nc.tensor.matmul(
                        eacc[0:2, 1, :], lhsE_sb[:, 2 * pb:2 * pb + 2], h2b[:],
                        start=True, stop=True)
                    st = stage.tile([2, 2, CHUNK], F32)
                    nc.vector.tensor_copy(st[:], eacc[0:2, :, :])
                    # st[c, u, n] -> out_t[4t + 2u + c, i*CHUNK + n]
                    dst = bass.AP(
                        tensor=out_t[:].tensor,
                        offset=(4 * t) * NC_SAMP + i * CHUNK,
                        ap=[[NC_SAMP, 2], [2 * NC_SAMP, 2], [1, CHUNK]],
                    )
                    nc.sync.dma_start(out=dst, in_=st[:])

    nc.compile()
    return nc


def _prep_weights(W_mix, W1, b1, W2, b2, W3):
    sp = np.logaddexp(0.0, W_mix.astype(np.float64))          # softplus, [D, L]
    W1e = W1.reshape(NPAIR, 2, H).astype(np.float64)          # [64, 2, 64]
    spe = sp.reshape(NPAIR, 2, L)                             # [64, 2, 16]
    # lhsA[l, p*128 + c*64 + h] = softplus(W_mix)[2p+c, l] * W1[2p+c, h]
    lhsA = np.einsum("pcl,pch->lpch", spe, W1e).astype(np.float32)
    lhsA = np.ascontiguousarray(lhsA.reshape(L, NPAIR * 128))

    blk = np.zeros((NPAIR, 128, 128), np.float32)
    blk[:, :H, :H] = W2[0::2]
    blk[:, H:, H:] = W2[1::2]
    lhs2 = np.ascontiguousarray(blk.transpose(1, 0, 2).reshape(128, NPAIR * 128))

    e = np.zeros((NPAIR, 128, 2), np.float32)
    e[:, :H, 0] = W3[0::2]
    e[:, H:, 1] = W3[1::2]
    lhsE = np.ascontiguousarray(e.transpose(1, 0, 2).reshape(128, NPAIR * 2))

    b1c = np.ascontiguousarray(
        np.concatenate([b1[0::2].T, b1[1::2].T], axis=0).astype(np.float32))
    b2c = np.ascontiguousarray(
        np.concatenate([b2[0::2].T, b2[1::2].T], axis=0).astype(np.float32))
    return lhsA, lhs2, lhsE, b1c, b2c


_NC_CACHE = None


def _get_nc():
    global _NC_CACHE
    if _NC_CACHE is None:
        _NC_CACHE = _build_bass()
    return _NC_CACHE


def kernel(z, W_mix, W1, b1, W2, b2, W3, b3):
    z = np.asarray(z, np.float32)
    lhsA, lhs2, lhsE, b1c, b2c = _prep_weights(
        np.asarray(W_mix), np.asarray(W1), np.asarray(b1), np.asarray(W2),
        np.asarray(b2), np.asarray(W3))

    nc = _get_nc()
    in_maps = []
    for c in range(N_CORES):
        zc = np.ascontiguousarray(z[c * NC_SAMP:(c + 1) * NC_SAMP, :].T)
        in_maps.append({
            "z_t": zc, "lhsA": lhsA, "lhs2": lhs2, "lhsE": lhsE,
            "b1c": b1c, "b2c": b2c,
        })
    res = run_bass_kernel_spmd(nc, in_maps, core_ids=list(range(N_CORES)))
    out = np.concatenate([r["out_t"].T for r in res.results], axis=0)
    out = out + np.asarray(b3, np.float32)[None, :]
    return np.ascontiguousarray(out.astype(np.float32))
